# revision 3
# baseline (speedup 1.0000x reference)
"""Trainium2 Bass kernel: per-sample position-decay mask multiply.

out[b, l, h] = data[b, l, h] * mask[b, l]
  mask[b, l] = 1 - (a_end - l)/C           if l < a_end
             = 1 - (l - a_idx)/C           elif l < sents_len
             = 0                           otherwise
  with a_end = aspect_Index + aspect_len, C = 40.

Memory-bound elementwise op; the kernel minimizes HBM bytes and then
balances the resulting compute across engines:

- Ragged skip: for l >= act = max(a_end, sents_len) the output is
  structurally zero; the host ships only the active prefix data[b, :act_b]
  and pre-zeroes the output buffer.
- int8 wire format BOTH ways: inputs linearly quantized to int8 on host
  (scale s_in), outputs returned int8 (scale s_out), dequantized on
  host. Device computes out_i8 = rne_sat(d_i8 * m') with the scaled
  mask m' = mask*s_in/s_out in fp16 (measured: DVE/ACT float->int8
  writes round-to-nearest-even and saturate). Total quantization error
  ~9e-3 of max|out| vs the 2e-2 budget. Halves HBM traffic vs fp16.
- The cost: any 8-bit operand puts the DVE TensorTensor in 1x mode
  (measured 1.04 ns/elem vs 0.52 at 2x), so a pure-DVE int8 kernel is
  DVE-bound at ~13.5us. Fix: 'b'-path tiles are cast i8->f16 by the
  ACT engine (~0.86 ns/elem), multiplied on the DVE in 2x packed-fp16
  mode, and cast back f16->i8 by ACT; 'a'-path tiles do the direct 1x
  int8 TT on the DVE. The b-fraction is tuned so DVE and ACT finish
  together (~11us each), inside the ~11us DMA window.
- Mask is pair-duplicated fp16 (m[2l]=m[2l+1]) embedded as raw bytes at
  the head of the int8 stream (bitcast view on SBUF): the 2x TT needs
  every operand's innermost dim stride-1 x >=2; 1x tiles view the same
  bytes with a stride-2 AP. Rides in with tile 0's load.
- Small first tile (starts the DVE chain ~0.4us earlier) and tiny last
  tiles (short store-drain tail). Loads split across both HWDGE rings;
  stores split so the ACT engine (which also runs the casts) issues
  only the late-tile stores. A dummy 1-col ACTIVATE is emitted first so
  the one-time ~1.3us ACT_TABLE_LOAD hides under the load phase.
- Samples are LPT-assigned to cores on act; PP is shared by all cores
  (SPMD: one compiled program). Compile cached by PP.
"""

import numpy as np

import concourse.bacc as bacc
import concourse.mybir as mybir
import concourse.tile as tile
from concourse.bass_utils import run_bass_kernel_spmd

N_CORES = 8
B, L, H = 512, 512, 100
C = 40.0
P = 128
HH = H // 2

F16 = mybir.dt.float16
I8 = mybir.dt.int8

# (width, path, load_ring, store_ring); rings: 's'=sync(SP), 'a'=scalar(ACT)
# DVE_ORDER / ACT casts are derived below. Tuned for PP=129.
TILES_129 = [
    (4,  'a', 's', 's'),
    (24, 'b', 'a', 'a'),
    (21, 'a', 's', 's'),
    (24, 'b', 'a', 'a'),
    (21, 'a', 's', 's'),
    (13, 'a', 's', 's'),
    (14, 'a', 's', 'a'),
    (6,  'a', 'a', 'a'),
    (2,  'a', 'a', 'a'),
]
DVE_ORDER_129 = [0, 2, 1, 4, 3, 5, 6, 7, 8]


def _tiles_for(PP):
    if PP == 129:
        return TILES_129, DVE_ORDER_129
    # generic fallback: pure 'a' path, 8 tiles, loads split, stores on ACT
    NT = min(8, PP)
    base = PP // NT
    widths = [base + (1 if i < PP - base * NT else 0) for i in range(NT)]
    tiles = [(w, 'a', 's' if i % 2 == 0 else 'a', 'a')
             for i, w in enumerate(widths)]
    return tiles, list(range(NT))


def build_bass(PP):
    """Build + compile the SPMD program for PP positions per partition."""
    nc = bacc.Bacc("TRN2", target_bir_lowering=False, debug=False)

    MC = 4 * PP  # pair-duplicated fp16 mask bytes per partition
    data = nc.dram_tensor("data", [P, MC + PP * H], I8, kind="ExternalInput")
    out = nc.dram_tensor("out", [P, PP * H], I8, kind="ExternalOutput")

    tiles_cfg, dve_order = _tiles_for(PP)
    widths = [t[0] for t in tiles_cfg]
    assert sum(widths) == PP, (widths, PP)
    offs = [sum(widths[:i]) for i in range(len(widths))]
    ring = {'s': nc.sync, 'a': nc.scalar}
    MUL = mybir.AluOpType.mult

    with tile.TileContext(nc) as tc:
        with tc.tile_pool(name="io", bufs=len(tiles_cfg) + 4) as io:
            # dummy ACTIVATE first: triggers the one-time ACT_TABLE_LOAD
            # early so it hides under the DMA load phase.
            dummy8 = io.tile([P, 4], I8, tag="dm8")
            dummy16 = io.tile([P, 4], F16, tag="dm16")
            nc.gpsimd.memset(dummy8[:], 0)
            nc.scalar.activation(out=dummy16[:], in_=dummy8[:],
                                 func=mybir.ActivationFunctionType.Copy)

            # loads (emission order = tile order per ring)
            tls = {}
            for i, (w, path, lr, sr) in enumerate(tiles_cfg):
                cols = (MC if i == 0 else 0) + w * H
                t = io.tile([P, cols], I8, tag="io")
                src0 = 0 if i == 0 else MC + offs[i] * H
                ring[lr].dma_start(t[:], data.ap()[:, src0:src0 + cols])
                tls[i] = t

            mask_t = tls[0][:, 0:MC].bitcast(F16)  # [P, 2*PP] pair-dup

            def mpair(i):
                off, w = offs[i], widths[i]
                return mask_t[:, 2 * off:2 * (off + w)].rearrange(
                    "p (l k) -> p l k", k=2)  # [P, w, 2]

            def d8_2d(i):
                w, skip = widths[i], (MC if i == 0 else 0)
                return tls[i][:, skip:skip + w * H]

            # b-path: ACT cast-in i8 -> f16 staging (emitted in tile order)
            stage = {}
            for i, (w, path, lr, sr) in enumerate(tiles_cfg):
                if path != 'b':
                    continue
                st = io.tile([P, w * H], F16, tag="st")
                nc.scalar.activation(out=st[:], in_=d8_2d(i),
                                     func=mybir.ActivationFunctionType.Copy)
                stage[i] = st

            # multiplies on the DVE in dve_order
            for i in dve_order:
                w, path = tiles_cfg[i][0], tiles_cfg[i][1]
                if path == 'a':
                    d3 = d8_2d(i).rearrange("p (l h) -> p l h", h=H)
                    m3 = mpair(i)[:, :, 0:1].broadcast_to([P, w, H])
                    nc.vector.tensor_tensor(out=d3, in0=d3, in1=m3, op=MUL)
                else:
                    d4 = stage[i][:].rearrange("p (l hh k) -> p l hh k",
                                               hh=HH, k=2)
                    m4 = mpair(i).unsqueeze(2).broadcast_to([P, w, HH, 2])
                    nc.vector.tensor_tensor(out=d4, in0=d4, in1=m4, op=MUL)

            # b-path: ACT cast-out f16 -> i8 back into the load tile
            for i in dve_order:
                if tiles_cfg[i][1] == 'b':
                    nc.scalar.activation(
                        out=d8_2d(i), in_=stage[i][:],
                        func=mybir.ActivationFunctionType.Copy)

            # stores (emission order = dve order per ring)
            for i in dve_order:
                off, w = offs[i], widths[i]
                sr = tiles_cfg[i][3]
                ring[sr].dma_start(out.ap()[:, off * H:(off + w) * H],
                                   d8_2d(i))

    nc.compile()
    return nc


_NC_CACHE = {}


def _get_nc(PP):
    if PP not in _NC_CACHE:
        _NC_CACHE[PP] = build_bass(PP)
    return _NC_CACHE[PP]


def plan_and_pack(data, aspect_Index, aspect_len, sents_len):
    """LPT-assign samples to cores, quantize to int8, pack each core's
    active prefixes into a flat stream [128, 4*PP + PP*H] int8: pair-
    duplicated fp16 scaled-mask bytes followed by the quantized data."""
    data = np.asarray(data, dtype=np.float32)
    a_idx = np.asarray(aspect_Index).astype(np.int64)
    a_end = a_idx + np.asarray(aspect_len).astype(np.int64)
    s_len = np.asarray(sents_len).astype(np.int64)
    act = np.maximum(a_end, s_len)

    order = np.argsort(-act, kind="stable")
    loads = np.zeros(N_CORES, dtype=np.int64)
    cores = [[] for _ in range(N_CORES)]
    for b in order:
        c = int(np.argmin(loads))
        loads[c] += act[b]
        cores[c].append(int(b))
    PP = max(1, -(-int(loads.max()) // P))

    i = np.arange(L, dtype=np.float32)[None, :]
    ae = a_end[:, None].astype(np.float32)
    ai = a_idx[:, None].astype(np.float32)
    maskf = np.where(i < ae, 1.0 - (ae - i) / C,
                     np.where(i < s_len[:, None], 1.0 - (i - ai) / C,
                              0.0)).astype(np.float32)

    # int8 scales; tight output bound via per-(b,l) row maxima
    s_in = float(np.abs(data).max()) / 127.0
    row_absmax = np.abs(data).max(axis=2)  # [B, L]
    s_out = float((np.abs(maskf) * row_absmax).max()) / 127.0
    if s_in <= 0:
        s_in = 1.0
    if s_out <= 0:
        s_out = 1.0
    d8full = np.clip(np.round(data * (1.0 / s_in)), -127, 127).astype(np.int8)
    maskp = (maskf * (s_in / s_out)).astype(np.float16)  # [B, L]

    in_maps, recon = [], []
    for c in range(N_CORES):
        mine = cores[c]
        S = int(act[mine].sum()) if mine else 0
        buf = np.zeros((P * PP, H), dtype=np.int8)
        mk = np.zeros((P * PP, 2), dtype=np.float16)
        off = 0
        for b in mine:
            a = int(act[b])
            buf[off:off + a] = d8full[b, :a]
            mk[off:off + a] = maskp[b, :a, None]  # pair-duplicated
            off += a
        mk_bytes = mk.reshape(P, 2 * PP).view(np.int8)  # [P, 4*PP]
        in_maps.append({"data": np.concatenate(
            [mk_bytes, buf.reshape(P, PP * H)], axis=1)})
        recon.append((mine, S))
    return in_maps, recon, PP, s_out


def kernel(data, aspect_Index, aspect_len, sents_len):
    in_maps, recon, PP, s_out = plan_and_pack(
        data, aspect_Index, aspect_len, sents_len)
    a_idx = np.asarray(aspect_Index).astype(np.int64)
    a_end = a_idx + np.asarray(aspect_len).astype(np.int64)
    act = np.maximum(a_end, np.asarray(sents_len).astype(np.int64))

    nc = _get_nc(PP)
    res = run_bass_kernel_spmd(nc, in_maps, list(range(N_CORES)))

    out = np.zeros((B, L, H), dtype=np.float32)
    for c in range(N_CORES):
        mine, S = recon[c]
        r = res.results[c]["out"].reshape(P * PP, H)[:S].astype(
            np.float32) * np.float32(s_out)
        off = 0
        for b in mine:
            a = int(act[b])
            out[b, :a] = r[off:off + a]
            off += a
    return out


if __name__ == "__main__":
    rng = np.random.default_rng(1)
    d = rng.standard_normal((B, L, H), dtype=np.float32)
    ai = rng.integers(0, 100, B).astype(np.int64)
    al = rng.integers(0, 10, B).astype(np.int64)
    slv = rng.integers(0, 512, B).astype(np.int64)
    got = kernel(d, ai, al, slv)
    i = np.arange(L, dtype=np.float32)[None, :]
    ae = (ai + al).astype(np.float32)[:, None]
    aif = ai.astype(np.float32)[:, None]
    m = np.where(i < ae, 1.0 - (ae - i) / C,
                 np.where(i < slv[:, None], 1.0 - (i - aif) / C, 0.0))
    want = d * m[:, :, None].astype(np.float32)
    err = np.abs(got - want)
    print("selftest max abs err:", err.max(),
          "rel:", err.max() / np.abs(want).max())


# revision 4
# speedup vs baseline: 1.0902x; 1.0902x over previous
"""Trainium2 Bass kernel: per-sample position-decay mask multiply.

out[b, l, h] = data[b, l, h] * mask[b, l]
  mask[b, l] = 1 - (a_end - l)/C           if l < a_end
             = 1 - (l - a_idx)/C           elif l < sents_len
             = 0                           otherwise
  with a_end = aspect_Index + aspect_len, C = 40.

Memory-bound elementwise op; the kernel minimizes HBM bytes and then
balances the resulting compute across engines:

- Ragged skip: for l >= act = max(a_end, sents_len) the output is
  structurally zero; the host ships only the active prefix data[b, :act_b]
  and pre-zeroes the output buffer.
- int8 wire format: inputs linearly quantized to int8 on host (scale
  s_in); most output tiles return int8 (scale s_out) and are dequantized
  on host. Device computes out_i8 = rne_sat(d_i8 * m') with the scaled
  mask m' = mask*s_in/s_out in fp16 (measured: DVE float->int8 writes
  round-to-nearest-even and saturate). Total quantization error ~9e-3
  of max|out| vs the 2e-2 budget. Nearly halves HBM traffic vs fp16.
- The cost: any 8-bit operand puts the DVE TensorTensor in 1x mode
  (measured 1.04 ns/elem vs 0.52 at 2x), so a pure-DVE int8 kernel is
  DVE-bound at ~13.5us. Fix: 'c'-path tiles are cast i8->f16 by the
  otherwise-idle ACT engine (~0.96 ns/elem), multiplied on the DVE in
  2x packed-fp16 mode, and stored as f16 (extra store bytes are free --
  the DMA rings have slack). 'a'-path tiles do the direct 1x int8 TT.
  The c fraction is tuned so DVE and ACT finish together.
- Mask is pair-duplicated fp16 (m[2l]=m[2l+1]) embedded as raw bytes at
  the head of the int8 stream (bitcast view on SBUF): the 2x TT needs
  every operand's innermost dim stride-1 x >=2; 1x tiles view the same
  bytes with a stride-2 AP. Rides in with tile 0's load.
- A dummy 1-col ACTIVATE is emitted first so the one-time ~1.3us
  ACT_TABLE_LOAD hides under the DMA load phase.
- Loads split across both HWDGE rings (tile 0 small and first on the SP
  ring: its completion gates the whole DVE chain); stores on the ACT
  ring; multiplies emitted in load-arrival order.
- Samples are LPT-assigned to cores on act; PP is shared by all cores
  (SPMD: one compiled program). Compile cached by PP.
"""

import numpy as np

import concourse.bacc as bacc
import concourse.mybir as mybir
import concourse.tile as tile
from concourse.bass_utils import run_bass_kernel_spmd

N_CORES = 8
B, L, H = 512, 512, 100
C = 40.0
P = 128
HH = H // 2

F16 = mybir.dt.float16
I8 = mybir.dt.int8

# tuned for PP=129: (width, path, load_ring); 's'=sync(SP), 'a'=scalar(ACT)
# c-path = ACT cast-in, DVE 2x TT, f16 store. DVE order interleaves c
# tiles after their casts complete.
TILES_129 = [
    (8,  'a', 's'),
    (24, 'c', 'a'),
    (21, 'a', 's'),
    (24, 'c', 'a'),
    (21, 'a', 's'),
    (13, 'a', 'a'),
    (10, 'a', 's'),
    (8,  'a', 'a'),
]
DVE_ORDER_129 = [0, 2, 1, 4, 3, 5, 6, 7]


def _tiles_for(PP):
    if PP == 129:
        return TILES_129, DVE_ORDER_129
    NT = min(8, PP)
    base = PP // NT
    widths = [base + (1 if i < PP - base * NT else 0) for i in range(NT)]
    tiles = [(w, 'a', 's' if i % 2 == 0 else 'a')
             for i, w in enumerate(widths)]
    return tiles, list(range(NT))


def _splits(PP):
    """(tiles_cfg, dve_order, offs, c_tiles, c_offs) for PP."""
    tiles_cfg, dve_order = _tiles_for(PP)
    widths = [t[0] for t in tiles_cfg]
    assert sum(widths) == PP, (widths, PP)
    offs = [sum(widths[:i]) for i in range(len(widths))]
    c_tiles = [i for i, t in enumerate(tiles_cfg) if t[1] == 'c']
    c_offs = {}
    acc = 0
    for i in c_tiles:
        c_offs[i] = acc
        acc += widths[i]
    return tiles_cfg, dve_order, offs, c_tiles, c_offs, acc


def build_bass(PP):
    """Build + compile the SPMD program for PP positions per partition."""
    nc = bacc.Bacc("TRN2", target_bir_lowering=False, debug=False)

    tiles_cfg, dve_order, offs, c_tiles, c_offs, c_total = _splits(PP)
    widths = [t[0] for t in tiles_cfg]

    MC = 4 * PP  # pair-duplicated fp16 mask bytes per partition
    data = nc.dram_tensor("data", [P, MC + PP * H], I8, kind="ExternalInput")
    out = nc.dram_tensor("out", [P, PP * H], I8, kind="ExternalOutput")
    out16 = None
    if c_total:
        out16 = nc.dram_tensor("out16", [P, c_total * H], F16,
                               kind="ExternalOutput")

    ring = {'s': nc.sync, 'a': nc.scalar}
    MUL = mybir.AluOpType.mult

    with tile.TileContext(nc) as tc:
        with tc.tile_pool(name="io", bufs=len(tiles_cfg) + 4) as io:
            if c_tiles:
                # dummy ACTIVATE: trigger one-time ACT_TABLE_LOAD early
                dummy8 = io.tile([P, 4], I8, tag="dm8")
                dummy16 = io.tile([P, 4], F16, tag="dm16")
                nc.gpsimd.memset(dummy8[:], 0)
                nc.scalar.activation(out=dummy16[:], in_=dummy8[:],
                                     func=mybir.ActivationFunctionType.Copy)

            tls = {}
            for i, (w, path, lr) in enumerate(tiles_cfg):
                cols = (MC if i == 0 else 0) + w * H
                t = io.tile([P, cols], I8, tag="io")
                src0 = 0 if i == 0 else MC + offs[i] * H
                ring[lr].dma_start(t[:], data.ap()[:, src0:src0 + cols])
                tls[i] = t

            mask_t = tls[0][:, 0:MC].bitcast(F16)  # [P, 2*PP] pair-dup

            def mpair(i):
                off, w = offs[i], widths[i]
                return mask_t[:, 2 * off:2 * (off + w)].rearrange(
                    "p (l k) -> p l k", k=2)  # [P, w, 2]

            def d8_2d(i):
                w, skip = widths[i], (MC if i == 0 else 0)
                return tls[i][:, skip:skip + w * H]

            # c-path: ACT cast-in i8 -> f16 staging (in tile order)
            stage = {}
            for i in c_tiles:
                st = io.tile([P, widths[i] * H], F16, tag="st")
                nc.scalar.activation(out=st[:], in_=d8_2d(i),
                                     func=mybir.ActivationFunctionType.Copy)
                stage[i] = st

            for i in dve_order:
                w, path = tiles_cfg[i][0], tiles_cfg[i][1]
                if path == 'a':
                    d3 = d8_2d(i).rearrange("p (l h) -> p l h", h=H)
                    m3 = mpair(i)[:, :, 0:1].broadcast_to([P, w, H])
                    nc.vector.tensor_tensor(out=d3, in0=d3, in1=m3, op=MUL)
                else:
                    d4 = stage[i][:].rearrange("p (l hh k) -> p l hh k",
                                               hh=HH, k=2)
                    m4 = mpair(i).unsqueeze(2).broadcast_to([P, w, HH, 2])
                    nc.vector.tensor_tensor(out=d4, in0=d4, in1=m4, op=MUL)

            for i in dve_order:
                off, w = offs[i], widths[i]
                if tiles_cfg[i][1] == 'c':
                    co = c_offs[i]
                    nc.scalar.dma_start(
                        out16.ap()[:, co * H:(co + w) * H], stage[i][:])
                else:
                    nc.scalar.dma_start(
                        out.ap()[:, off * H:(off + w) * H], d8_2d(i))

    nc.compile()
    return nc


_NC_CACHE = {}


def _get_nc(PP):
    if PP not in _NC_CACHE:
        _NC_CACHE[PP] = build_bass(PP)
    return _NC_CACHE[PP]


def plan_and_pack(data, aspect_Index, aspect_len, sents_len):
    """LPT-assign samples to cores, quantize to int8, pack each core's
    active prefixes into a flat stream [128, 4*PP + PP*H] int8: pair-
    duplicated fp16 scaled-mask bytes followed by the quantized data."""
    data = np.asarray(data, dtype=np.float32)
    a_idx = np.asarray(aspect_Index).astype(np.int64)
    a_end = a_idx + np.asarray(aspect_len).astype(np.int64)
    s_len = np.asarray(sents_len).astype(np.int64)
    act = np.maximum(a_end, s_len)

    order = np.argsort(-act, kind="stable")
    loads = np.zeros(N_CORES, dtype=np.int64)
    cores = [[] for _ in range(N_CORES)]
    for b in order:
        c = int(np.argmin(loads))
        loads[c] += act[b]
        cores[c].append(int(b))
    PP = max(1, -(-int(loads.max()) // P))

    i = np.arange(L, dtype=np.float32)[None, :]
    ae = a_end[:, None].astype(np.float32)
    ai = a_idx[:, None].astype(np.float32)
    maskf = np.where(i < ae, 1.0 - (ae - i) / C,
                     np.where(i < s_len[:, None], 1.0 - (i - ai) / C,
                              0.0)).astype(np.float32)

    # int8 scales; tight output bound via per-(b,l) row maxima
    s_in = float(np.abs(data).max()) / 127.0
    row_absmax = np.abs(data).max(axis=2)  # [B, L]
    s_out = float((np.abs(maskf) * row_absmax).max()) / 127.0
    if s_in <= 0:
        s_in = 1.0
    if s_out <= 0:
        s_out = 1.0
    d8full = np.clip(np.round(data * (1.0 / s_in)), -127, 127).astype(np.int8)
    maskp = (maskf * (s_in / s_out)).astype(np.float16)  # [B, L]

    in_maps, recon = [], []
    for c in range(N_CORES):
        mine = cores[c]
        S = int(act[mine].sum()) if mine else 0
        buf = np.zeros((P * PP, H), dtype=np.int8)
        mk = np.zeros((P * PP, 2), dtype=np.float16)
        off = 0
        for b in mine:
            a = int(act[b])
            buf[off:off + a] = d8full[b, :a]
            mk[off:off + a] = maskp[b, :a, None]  # pair-duplicated
            off += a
        mk_bytes = mk.reshape(P, 2 * PP).view(np.int8)  # [P, 4*PP]
        in_maps.append({"data": np.concatenate(
            [mk_bytes, buf.reshape(P, PP * H)], axis=1)})
        recon.append((mine, S))
    return in_maps, recon, PP, s_out


def kernel(data, aspect_Index, aspect_len, sents_len):
    in_maps, recon, PP, s_out = plan_and_pack(
        data, aspect_Index, aspect_len, sents_len)
    a_idx = np.asarray(aspect_Index).astype(np.int64)
    a_end = a_idx + np.asarray(aspect_len).astype(np.int64)
    act = np.maximum(a_end, np.asarray(sents_len).astype(np.int64))

    nc = _get_nc(PP)
    res = run_bass_kernel_spmd(nc, in_maps, list(range(N_CORES)))

    tiles_cfg, _, offs, c_tiles, c_offs, c_total = _splits(PP)
    widths = [t[0] for t in tiles_cfg]

    out = np.zeros((B, L, H), dtype=np.float32)
    for c in range(N_CORES):
        mine, S = recon[c]
        r8 = res.results[c]["out"].reshape(P, PP * H)
        # stitch per-position stream [P*PP, H] from the two outputs
        full = r8.astype(np.float32)
        if c_total:
            r16 = res.results[c]["out16"].reshape(P, c_total * H)
            for i in c_tiles:
                off, w, co = offs[i], widths[i], c_offs[i]
                full[:, off * H:(off + w) * H] = \
                    r16[:, co * H:(co + w) * H].astype(np.float32)
        r = full.reshape(P * PP, H)[:S] * np.float32(s_out)
        off = 0
        for b in mine:
            a = int(act[b])
            out[b, :a] = r[off:off + a]
            off += a
    return out


if __name__ == "__main__":
    rng = np.random.default_rng(1)
    d = rng.standard_normal((B, L, H), dtype=np.float32)
    ai = rng.integers(0, 100, B).astype(np.int64)
    al = rng.integers(0, 10, B).astype(np.int64)
    slv = rng.integers(0, 512, B).astype(np.int64)
    got = kernel(d, ai, al, slv)
    i = np.arange(L, dtype=np.float32)[None, :]
    ae = (ai + al).astype(np.float32)[:, None]
    aif = ai.astype(np.float32)[:, None]
    m = np.where(i < ae, 1.0 - (ae - i) / C,
                 np.where(i < slv[:, None], 1.0 - (i - aif) / C, 0.0))
    want = d * m[:, :, None].astype(np.float32)
    err = np.abs(got - want)
    print("selftest max abs err:", err.max(),
          "rel:", err.max() / np.abs(want).max())
